# revision 12
# baseline (speedup 1.0000x reference)
"""Trainium2 Bass kernel for nn_CascadeGradNetOURS (dense_mlp).

Math (reference):
    h = x @ W.T                       # (B, E), shared by all layers
    z = beta[0] * (h + b[0])
    for i in 0..6:
        z = beta[i+1]*(h + b[i+1]) + alpha[i]*relu(z)
    z = alpha[7] * relu(z)
    out = z @ W + bias_last           # (B, IN)

Device formulation (per core, batch-sharded 1024 rows, transposed layout
hT[e, b] so per-layer alpha/beta/bias become per-PARTITION scalars):

Track y_k = gamma_k z_k (+ d_1 shift at k=1 only), with gamma_1 = 1/beta0
(y_1 = h) and gamma_k = sign(alpha[k-2]) for k>=2. Each layer is then
    q_k     = max(m_k * y_k, t_k)            [tensor_scalar mult+max, 4x DVE]
            = |alpha[k-1]| relu(z_k) + e_k
    ht_k    = a_k * h + c_k                  [per-partition affine: ACT/DVE/GPS]
    y_{k+1} = ht_k + q_k                     [tensor_tensor add: DVE/GPS]
with t_k = e_k = 0 for k>=2 (the c_k bias re-zeros the shift each layer).
q_8 = |alpha7| relu(z_8); sign(alpha7) is folded into W2 rows on host.

This replaces the old ACT-relu + 1x-mode scalar_tensor_tensor cascade with
4x-mode tensor_scalar + 2x-mode tensor_tensor ops, spread across ACT, DVE
and the otherwise-idle GpSimd engine so the elementwise work hides under
the ~265us of PE matmul (1024 N=512 MMs @ ~259ns).

All matmul/cascade tensors fp16 (PSUM accumulates fp32).
"""

import os

os.environ.setdefault("MYCRO_LOCAL_CACHE", "1")

import numpy as np

import concourse.bacc as bacc
import concourse.bass as bass
import concourse.mybir as mybir
from concourse.tile import TileContext

N_CORES = 8
B, IN, E, L = 8192, 1024, 4096, 8
BC = B // N_CORES          # 1024 batch rows per core
NI = IN // 128             # 8 i-chunks
NE = E // 128              # 32 e-chunks
F16 = mybir.dt.float16
F32 = mybir.dt.float32
NCONST = 24

GROUP = 4                  # e-chunks interleaved in the cascade pipeline
W1ECS = 20                 # mm2 window-1 depth (overlapped under the cascade)

# Engine assignment for the cascade, tunable. Keys are layer k=1..7.
# NOTE: GpSimd ("GPS") is a trap here — DVE and GpSimd arbitrate a shared
# SBUF port pair (lock held per instruction, loser fully blocks), so GPS
# streaming ops serialize against every multi-port DVE op. Measured: each
# concurrent GPS op inflated a DVE op to the GPS op's duration. ACT+DVE only.
# "DMA" tt-adds ride the CCE accumulate path (dst += src in the DMA engines,
# AXI ports — physically can't contend with ACT/DVE SBUF ports), issued from
# the mostly-idle Sync engine's HWDGE queue.
HT_ENG = {1: "ACT", 2: "ACT", 3: "ACT", 4: "ACT", 5: "DVE", 6: "DVE", 7: "DVE"}
TT_ENG = {1: "DVE", 2: "DMA", 3: "DVE", 4: "DMA", 5: "DVE", 6: "DMA", 7: "DVE"}

# consts columns
C_M = 0          # cols 0..7:   m_k, k=1..8
C_T1 = 8         # col 8:       t_1 (R1 max threshold)
C_A = 9          # cols 9..15:  a_k, k=1..7
C_C = 16         # cols 16..22: c_k, k=1..7


_SEQ_ONLY = {
    "InstUnconditionalBranch",
    "InstCall",
    "InstISA",
}


def _legalize_waits(nc):
    """Datapath instructions carry exactly ONE semaphore wait slot in the
    64-byte ISA encoding (walrus errors on more). Engine sequencers execute
    their stream in order, so any extra waits can be hoisted onto single-wait
    NoOps inserted immediately before the capped instruction — semantically
    identical (all waits still complete before the instruction executes).
    For HWDGE DMAs prefer keeping a DMA-queue wait in-descriptor and hoist
    engine-sem waits to the sequencer."""
    import bass_rust

    uid = 0
    for bb in nc.m.functions[0].blocks:
        insts = bb.instructions  # live list
        newlist = []
        for i in insts:
            cls = i.__class__.__name__
            si = i.sync_info
            if cls in _SEQ_ONLY or si is None or len(si.on_wait) <= 1:
                newlist.append(i)
                continue
            waits = list(si.on_wait)
            if cls == "InstDMACopy":
                dmaw = [w for w in waits if w.ant_name.startswith("DMA")]
                keep = dmaw[-1] if dmaw else waits[-1]
            else:
                keep = waits[-1]
            rest = [w for w in waits if w is not keep]
            for w in rest:
                uid += 1
                nop = mybir.InstNoOp(
                    name=f"waitnop-{uid}-{i.name}",
                    engine=i.engine,
                    bass_nofuse=True,
                )
                nop.sync_info = bass_rust.SyncInfo(on_wait=[w], on_update=[])
                newlist.append(nop)
            si.on_wait = [keep]
            newlist.append(i)
        if len(newlist) != len(insts):
            insts[:] = newlist


def build_nc() -> bass.Bass:
    nc = bacc.Bacc()
    AL = mybir.AluOpType
    AF = mybir.ActivationFunctionType

    xTd = nc.declare_dram_parameter("xT", [128, NI, BC], F16, isOutput=False)
    WTd = nc.declare_dram_parameter("WT", [128, NE, NI, 128], F16, isOutput=False)
    W2d = nc.declare_dram_parameter("W2", [128, NE, IN], F16, isOutput=False)
    Cd = nc.declare_dram_parameter("consts", [128, NE, NCONST], F32, isOutput=False)
    Bd = nc.declare_dram_parameter("blast", [128, NI], F32, isOutput=False)
    Od = nc.declare_dram_parameter("outT", [128, NI, BC], F32, isOutput=True)

    with TileContext(nc) as tc:
        with (
            tc.tile_pool(name="persist", bufs=1) as persist,
            tc.tile_pool(name="wtp", bufs=2) as wtp,
            tc.tile_pool(name="work", bufs=1) as work,
            tc.tile_pool(name="outp", bufs=1) as outp,
            tc.tile_pool(name="psum_h", bufs=2, space="PSUM") as psum_h,
            tc.tile_pool(name="psum_o", bufs=3, space="PSUM") as psum_o,
        ):
            consts_sb = persist.tile([128, NE, NCONST], F32)
            nc.sync.dma_start(out=consts_sb, in_=Cd[:, :, :])
            blast_sb = persist.tile([128, NI], F32)
            nc.sync.dma_start(out=blast_sb, in_=Bd[:, :])
            x_sb = persist.tile([128, NI, BC], F16)
            for i in range(NI):
                nc.sync.dma_start(out=x_sb[:, i, :], in_=xTd[:, i, :])
            w2_sb = persist.tile([128, NE, IN], F16)
            z_sb = persist.tile([128, NE, BC], F16)
            o_acc = persist.tile([128, NI, BC], F16)

            def c_ap(ec, col):
                return consts_sb[:, ec, col : col + 1]

            def emit_mm2_group(ic, hf, ec_lo, ec_hi, mode):
                ops = psum_o.tile(
                    [128, 512], F32, tag="o", name=f"o_{ic}_{hf}_{ec_lo}"
                )
                for ec in range(ec_lo, ec_hi):
                    nc.tensor.matmul(
                        ops,
                        w2_sb[:, ec, ic * 128 : (ic + 1) * 128],
                        z_sb[:, ec, hf * 512 : (hf + 1) * 512],
                        start=(ec == ec_lo),
                        stop=(ec == ec_hi - 1),
                    )
                bsl = hf * 512
                if mode == "acc_init":
                    # bias_last folded here; fp16 partial staging
                    nc.scalar.activation(
                        out=o_acc[:, ic, bsl : bsl + 512],
                        in_=ops,
                        func=AF.Identity,
                        bias=blast_sb[:, ic : ic + 1],
                        scale=1.0,
                    )
                elif mode == "acc_add":
                    nc.vector.tensor_tensor(
                        out=o_acc[:, ic, bsl : bsl + 512],
                        in0=o_acc[:, ic, bsl : bsl + 512], in1=ops, op=AL.add,
                    )
                else:
                    osb = outp.tile([128, 512], F32, tag="osb")
                    nc.vector.tensor_tensor(
                        out=osb, in0=o_acc[:, ic, bsl : bsl + 512], in1=ops,
                        op=AL.add,
                    )
                    nc.scalar.dma_start(
                        out=Od[:, ic, bsl : bsl + 512], in_=osb
                    )

            # window-1 group ids spread over phase-A tail groups
            w1_sched = {20: range(0, 5), 24: range(5, 10), 28: range(10, 16)}

            def eng(name):
                return {"ACT": nc.scalar, "DVE": nc.vector, "GPS": nc.gpsimd}[name]

            # ---------------- Phase A: mm1 + cascade ----------------
            for g0 in range(0, NE, GROUP):
                if g0 == 2 * GROUP:
                    # w2 is first needed in phase B; keep it off the startup
                    # DMA critical path but well ahead of mm2.
                    for g in range(4):
                        nc.sync.dma_start(
                            out=w2_sb[:, g * 8 : (g + 1) * 8, :],
                            in_=W2d[:, g * 8 : (g + 1) * 8, :],
                        )
                for gi in w1_sched.get(g0, ()):
                    emit_mm2_group(gi // 2, gi % 2, 0, W1ECS, mode="acc_init")
                ecs = range(g0, g0 + GROUP)
                h_ps = {}
                for ec in ecs:
                    wt = wtp.tile([128, NI, 128], F16, tag="wt")
                    nc.sync.dma_start(out=wt, in_=WTd[:, ec, :, :])
                    hp = psum_h.tile([128, BC], F32, tag="h")
                    for i in range(NI):
                        lhsT = wt[:, i, :]
                        for hf in range(2):
                            nc.tensor.matmul(
                                hp[:, hf * 512 : (hf + 1) * 512],
                                lhsT,
                                x_sb[:, i, hf * 512 : (hf + 1) * 512],
                                start=(i == 0),
                                stop=(i == NI - 1),
                            )
                    h_ps[ec] = hp
                hsb = {}
                for ec in ecs:
                    t = work.tile([128, BC], F16, tag="hsb", bufs=GROUP + 2, name=f"hsb_{ec}")
                    nc.scalar.copy(out=t, in_=h_ps[ec])
                    hsb[ec] = t
                # R1: q_1 = max(m_1 * h, t_1)   [DVE ts 4x, from h_sb fp16]
                q = {}
                for ec in ecs:
                    qt = work.tile(
                        [128, BC], F16, tag="q", bufs=GROUP + 2, name=f"q_{ec}_1"
                    )
                    nc.vector.tensor_scalar(
                        qt, hsb[ec], c_ap(ec, C_M + 0), c_ap(ec, C_T1),
                        AL.mult, AL.max,
                    )
                    q[ec] = qt
                # cascade, layer-major across the group for cross-engine overlap
                for k in range(1, L):       # k = 1..7
                    for ec in ecs:
                        ht = work.tile(
                            [128, BC], F16, tag="ht", bufs=4, name=f"ht_{ec}_{k}"
                        )
                        e_ht = eng(HT_ENG[k])
                        if HT_ENG[k] == "ACT":
                            nc.scalar.activation(
                                out=ht, in_=hsb[ec], func=AF.Identity,
                                bias=c_ap(ec, C_C + k - 1),
                                scale=c_ap(ec, C_A + k - 1),
                            )
                        else:
                            e_ht.tensor_scalar(
                                ht, hsb[ec], c_ap(ec, C_A + k - 1),
                                c_ap(ec, C_C + k - 1), AL.mult, AL.add,
                            )
                        if TT_ENG[k] == "DMA":
                            # CCE accumulate: ht += q in the DMA engines; the
                            # ht tile becomes y_{k+1} in place. Accum DMAs are
                            # SWDGE-only (descriptor gen on the Q7 cores).
                            nc.gpsimd.dma_start(out=ht, in_=q[ec], accum_op=AL.add)
                            yt = ht
                        else:
                            yt = work.tile(
                                [128, BC], F16, tag="y", bufs=3, name=f"y_{ec}_{k}"
                            )
                            eng(TT_ENG[k]).tensor_tensor(
                                out=yt, in0=ht, in1=q[ec], op=AL.add
                            )
                        if k < L - 1:
                            qt = work.tile(
                                [128, BC], F16, tag="q", bufs=GROUP + 2,
                                name=f"q_{ec}_{k + 1}",
                            )
                            nc.vector.tensor_scalar(
                                qt, yt, c_ap(ec, C_M + k), 0.0, AL.mult, AL.max
                            )
                            q[ec] = qt
                        else:
                            # R8 on ACT: z = relu(m_8 * y_8)
                            nc.scalar.activation(
                                out=z_sb[:, ec, :], in_=yt, func=AF.Relu,
                                bias=0.0, scale=c_ap(ec, C_M + L - 1),
                            )

            # ---------------- Phase B: mm2 windows 2+3 ----------------
            # w2a (8 ec) overlaps the cascade tail; w2b (4 ec) is the only
            # work gated on the final cascade group, minimizing the PE tail.
            for gi in range(16):
                emit_mm2_group(gi // 2, gi % 2, W1ECS, 28, mode="acc_add")
            for gi in range(16):
                emit_mm2_group(gi // 2, gi % 2, 28, NE, mode="final")

    nc.compile()
    return nc


def _prep_inputs(x, W, biases, bias_last, alpha, beta):
    """Host-side shard/relayout/constant precompute. Returns per-core in_maps."""
    x = np.asarray(x, np.float32)
    W = np.asarray(W, np.float32)
    biases = np.asarray(biases, np.float32)
    bias_last = np.asarray(bias_last, np.float32)
    alpha = np.asarray(alpha, np.float32)
    beta = np.asarray(beta, np.float32)

    b = biases
    s = np.where(alpha >= 0, 1.0, -1.0).astype(np.float32)  # s[j] = sign(alpha[j])
    m = np.zeros((L + 1, E), np.float32)                    # m[k], k=1..8
    m[1] = np.abs(alpha[0]) * beta[0]
    for k in range(2, L + 1):
        m[k] = np.abs(alpha[k - 1]) * s[k - 2]
    t1 = -m[1] * b[0]
    a_ = np.zeros((L, E), np.float32)                       # a[k], c[k], k=1..7
    c_ = np.zeros((L, E), np.float32)
    for k in range(1, L):
        a_[k] = s[k - 1] * beta[k]
        c_[k] = s[k - 1] * beta[k] * b[k]
    c_[1] = c_[1] - t1                                      # absorb e_1 = t_1

    consts = np.zeros((E, NCONST), np.float32)
    for k in range(1, L + 1):
        consts[:, C_M + k - 1] = m[k]
    consts[:, C_T1] = t1
    for k in range(1, L):
        consts[:, C_A + k - 1] = a_[k]
        consts[:, C_C + k - 1] = c_[k]
    consts_t = np.ascontiguousarray(
        consts.reshape(NE, 128, NCONST).transpose(1, 0, 2)
    )

    WT_t = np.ascontiguousarray(
        W.T.reshape(NI, 128, NE, 128).transpose(1, 2, 0, 3).astype(np.float16)
    )
    W2 = W * s[L - 1][:, None]
    W2_t = np.ascontiguousarray(
        W2.reshape(NE, 128, IN).transpose(1, 0, 2).astype(np.float16)
    )
    blast_t = np.ascontiguousarray(bias_last.reshape(NI, 128).T)

    in_maps = []
    for c in range(N_CORES):
        xc = x[c * BC : (c + 1) * BC]           # (BC, IN)
        xT = np.ascontiguousarray(
            xc.T.reshape(NI, 128, BC).transpose(1, 0, 2).astype(np.float16)
        )
        in_maps.append(
            {
                "xT": xT,
                "WT": WT_t,
                "W2": W2_t,
                "consts": consts_t,
                "blast": blast_t,
            }
        )
    return in_maps


_NC_CACHE = None


def _install_ntff_hook():
    """The agent image's antenv lacks axon_hooks; rebuild it from the boot
    helper so run_bass_kernel_spmd(trace=True) can capture NTFF profiles."""
    import sys
    import types

    if "antenv.axon_hooks" in sys.modules:
        return
    try:
        from trn_agent_boot.trn_boot import _ntff_profile_via_ctypes

        hook = _ntff_profile_via_ctypes("/opt/axon/libaxon_pjrt.so")
    except Exception:
        hook = None
    m = types.ModuleType("antenv.axon_hooks")
    m.get_axon_ntff_profile_hook = lambda: hook
    m.set_axon_ntff_profile_hook = lambda h: None
    sys.modules["antenv.axon_hooks"] = m


def run(inputs: dict, trace: bool = False):
    """Returns (out, BassKernelResults)."""
    global _NC_CACHE
    from concourse.bass_utils import run_bass_kernel_spmd

    if trace:
        _install_ntff_hook()

    if _NC_CACHE is None:
        _NC_CACHE = build_nc()
    nc = _NC_CACHE
    in_maps = _prep_inputs(**inputs)
    res = run_bass_kernel_spmd(nc, in_maps, list(range(N_CORES)), trace=trace)
    out = np.empty((B, IN), np.float32)
    for c in range(N_CORES):
        oc = np.asarray(res.results[c]["outT"])          # (128, NI, BC)
        out_core = oc.transpose(1, 0, 2).reshape(IN, BC) # (IN, BC) = outT
        out[c * BC : (c + 1) * BC] = out_core.T
    return out, res


def kernel(x, W, biases, bias_last, alpha, beta) -> np.ndarray:
    out, _ = run(
        dict(x=x, W=W, biases=biases, bias_last=bias_last, alpha=alpha, beta=beta)
    )
    return out
